# revision 80
# baseline (speedup 1.0000x reference)
"""DiSAN forward kernel on 8 TRN2 NeuronCores (Bass/Tile, SPMD).

Sharding: core c handles batch b = c//2 and query half c%2 (100 queries each).
Per-core token permutation (natural order for even cores, fully reversed for
odd ones) puts the core's queries at positions 0..99 and turns both attention
directions into the position windows [0,lq) / (lq,200), so one program serves
all 8 cores; the fw/bw meaning of the two branches is unscrambled on the host
by swapping weight feature-halves and output halves for odd cores.

The [L,L,D] attention tensor never touches HBM. The O(L*D) prologue
(h = elu(emb[x] @ Wh + b), h1 = (h@W1)/C, h2b = (h@W2+b)/C, hmean) runs on
the host alongside the embedding gather and ships as small packs. On device,
per group of G queries: one Pool add builds the logits (h2b broadcast over
the group + h1 column broadcast over keys; the 1/C tanh prescale is folded
in on the host), ScalarE runs one grouped tanh and one grouped exp (bf16),
then two bf16 product passes - ez = e*zq (zq = per-query effective key
mask, broadcast-DMAd from a host row: key-mask for real queries, all-ones
for pad queries) and ezh = ez*h - split between DVE tensor_tensor (2x mode)
and Pool to balance the engines. Per query, four windowed tensor_scalar
ops with accum_out (DVE 4x mode: all-bf16 packed SBUF operands) reduce
ez/ezh over the compile-time F/B window slices into num/den columns.
Queries with an empty key set (host-detected) carry fb=1; their s falls
back to mean(h), matching the reference's uniform softmax over an all
-1e13 row. The fusion-gate + source2token epilogue runs in two query
parts (75/25) so the first part overlaps the remaining loop without its
ACT ops delaying the loop's exp stream too early; each core
emits per-part partial poolings [D,6] and the host sums them and applies
the tiny final MLP.
"""

import numpy as np
import ml_dtypes
from contextlib import ExitStack

import concourse.bass as bass
import concourse.bacc as bacc
import concourse.tile as tile
from concourse import mybir
from concourse.bass_utils import run_bass_kernel_spmd

B, L, D, NCLS = 4, 200, 100, 20
Q = 100           # queries per core
NCORES = 8
CVAL = 5.0
F32 = mybir.dt.float32
BF16 = mybir.dt.bfloat16
AF = mybir.ActivationFunctionType
ALU = mybir.AluOpType

_CACHE = {}

# query-group sizes for the main loop (sum = Q); ramped ends so the first
# DVE/Pool work arrives early and the last group drains quickly. Queries
# in each finished part get their epilogue emitted early, overlapping the
# remaining loop. Within each group the ez/ezh product passes are split
# ~25% DVE (tensor_tensor 2x bf16, ~0.52ns/el) / 75% Pool (~0.83ns/el) so
# the per-group flows through DVE and Pool match; the logits adds always
# run on Pool (their h1-broadcast operand has innermost stride 0, which
# disqualifies the DVE fast modes) and DVE carries the fixed
# windowed-reduction load.
GROUPS = (1, 2, 3, 6, 10, 13, 15, 13, 12, 13, 12)
# epilogue part sizes; parts end at query boundaries 75 / 100, which
# must coincide with group boundaries so each part's epilogue can be
# emitted as soon as its queries finish
PARTS = (75, 25)
# per-group count of product-pass queries handled by DVE (rest on Pool)
KDS = (1, 2, 3, 6, 8, 6, 3, 3, 2, 2, 1)

# f32 packs of host-computed per-core activations. hf1 (h1s | h2bs) heads
# the whole pipeline, so it ships separately from hf2 (hT | hmean), which
# is only needed by the later product/epilogue stages.
PH1 = dict(H1=0, H2B=Q)
PH1_W = Q + L
PH2 = dict(H=0, HM=L)
PH2_W = L + 1
# f32 late pack: gate + source2token weights
PL = dict(WF1=0, WF2=100, WS1_0=200, WS1_1=400, WS_0=600, WS_1=800,
          WF2B=1000, WS1B=1001, WSB=1003, WF2BN=1005)
PL_W = 1006


def _free_bcast(ap, n):
    """Broadcast a [P,1] AP along the free dim to [P,n] with stride 0."""
    return bass.AP(tensor=ap.tensor, offset=ap.offset, ap=[ap.ap[0], [0, n]])


def _build_program():
    nc = bacc.Bacc()
    d_hb = nc.declare_dram_parameter("hb", [D, L], BF16, isOutput=False)
    d_hf1 = nc.declare_dram_parameter("hf1", [D, PH1_W], F32, isOutput=False)
    d_hf2 = nc.declare_dram_parameter("hf2", [D, PH2_W], F32, isOutput=False)
    d_late = nc.declare_dram_parameter("late", [D, PL_W], F32, isOutput=False)
    # per-query effective key mask rows: ones if query is PAD else key-mask
    d_zq = nc.declare_dram_parameter("zq", [1, Q * L], BF16, isOutput=False)
    # fb row: fbF | fbP  ({0,1} f32)
    d_fb = nc.declare_dram_parameter("fb", [1, 2 * Q], F32, isOutput=False)
    # per-part partial source2token poolings: [F_p0, P_p0, F_p1, P_p1, ...]
    d_out = nc.declare_dram_parameter("out", [D, 6], F32, isOutput=True)

    with tile.TileContext(nc) as tc, ExitStack() as ctx:
        singles = ctx.enter_context(tc.tile_pool(name="singles", bufs=1))
        work = ctx.enter_context(tc.tile_pool(name="work", bufs=3))
        psum = ctx.enter_context(tc.tile_pool(name="psum", bufs=4, space="PSUM"))
        tpool = ctx.enter_context(tc.tile_pool(name="tpool", bufs=3))
        apool = ctx.enter_context(tc.tile_pool(name="apool", bufs=3))
        epool = ctx.enter_context(tc.tile_pool(name="epool", bufs=4))
        zpool = ctx.enter_context(tc.tile_pool(name="zpool", bufs=3))
        ppool = ctx.enter_context(tc.tile_pool(name="ppool", bufs=4))
        scr = ctx.enter_context(tc.tile_pool(name="scr", bufs=3))

        # hf1 first: the h1s/h2bs it carries head the whole pipeline
        t_hf1 = singles.tile([D, PH1_W], F32, tag="hf1")
        nc.sync.dma_start(out=t_hf1[:], in_=d_hf1[:])
        t_hbt = singles.tile([D, L], BF16, tag="hbt")
        nc.sync.dma_start(out=t_hbt[:], in_=d_hb[:])
        t_hf2 = singles.tile([D, PH2_W], F32, tag="hf2")
        nc.sync.dma_start(out=t_hf2[:], in_=d_hf2[:])
        # fbF | fbP rows broadcast across partitions
        t_fb3 = singles.tile([D, 2 * Q], F32, tag="fb3")
        nc.sync.dma_start(out=t_fb3[:], in_=bass.AP(
            tensor=d_fb[:].tensor, offset=0, ap=[[0, D], [1, 2 * Q]]))
        t_late = singles.tile([D, PL_W], F32, tag="late")
        nc.gpsimd.dma_start(out=t_late[:], in_=d_late[:])

        t_h = t_hf2[:, PH2["H"]:PH2["H"] + L]
        t_h1 = t_hf1[:, PH1["H1"]:PH1["H1"] + Q]
        t_h2b = t_hf1[:, PH1["H2B"]:PH1["H2B"] + L]
        t_hm = t_hf2[:, PH2["HM"]:PH2["HM"] + 1]
        t_Wf1 = t_late[:, PL["WF1"]:PL["WF1"] + D]
        t_Wf2 = t_late[:, PL["WF2"]:PL["WF2"] + D]
        t_Ws1_0 = t_late[:, PL["WS1_0"]:PL["WS1_0"] + 2 * D]
        t_Ws1_1 = t_late[:, PL["WS1_1"]:PL["WS1_1"] + 2 * D]
        t_Ws_0 = t_late[:, PL["WS_0"]:PL["WS_0"] + 2 * D]
        t_Ws_1 = t_late[:, PL["WS_1"]:PL["WS_1"] + 2 * D]
        t_Wf2b = t_late[:, PL["WF2B"]:PL["WF2B"] + 1]
        t_Ws1b = t_late[:, PL["WS1B"]:PL["WS1B"] + 2]
        t_Wsb = t_late[:, PL["WSB"]:PL["WSB"] + 2]
        t_Wf2bn = t_late[:, PL["WF2BN"]:PL["WF2BN"] + 1]

        t_ones = singles.tile([1, D], F32)
        nc.vector.memset(t_ones[:], 1.0)
        # warm the ACT function-set table load (1.3us) during the input DMAs
        t_warm = singles.tile([1, 1], F32, tag="warm")
        nc.scalar.activation(t_warm[:], t_ones[0:1, 0:1], AF.Exp)

        # separate accumulator tiles per epilogue part so each part's
        # epilogue only depends on the groups that wrote it
        pof = [0]
        for w in PARTS:
            pof.append(pof[-1] + w)
        t_numF, t_denF, t_numB, t_denB = {}, {}, {}, {}
        for h, w in enumerate(PARTS):
            t_numF[h] = singles.tile([D, w], F32, name=f"t_numF{h}")
            t_denF[h] = singles.tile([D, w], F32, name=f"t_denF{h}")
            t_numB[h] = singles.tile([D, w], F32, name=f"t_numB{h}")
            t_denB[h] = singles.tile([D, w], F32, name=f"t_denB{h}")

        # zero the columns that sliced-window skipping never writes
        nc.gpsimd.memset(t_numB[0][:, 0:1], 0.0)
        nc.gpsimd.memset(t_denB[0][:, 0:1], 0.0)

        h2b_src = t_h2b
        hb_src = t_hbt[:]

        def emit_add(lq0, G):
            # t[d, k, m] = h2bs[d, m] + h1s[d, lq0+k]  (Pool)
            h2b_grp = bass.AP(tensor=h2b_src.tensor, offset=h2b_src.offset,
                              ap=[h2b_src.ap[0], [0, G], [1, L]])
            h1c = t_h1[:, lq0:lq0 + G]
            h1_grp = bass.AP(tensor=h1c.tensor, offset=h1c.offset,
                             ap=[h1c.ap[0], h1c.ap[1], [0, L]])
            t_t = tpool.tile([D, G, L], F32, tag="t", name=f"t{lq0}")
            nc.gpsimd.tensor_add(t_t[:], h2b_grp, h1_grp)
            return t_t

        def emit_zq_dma(lq0, G):
            t_zq = zpool.tile([D, G, L], BF16, tag="zq", name=f"zq{lq0}")
            nc.sync.dma_start(out=t_zq[:], in_=bass.AP(
                tensor=d_zq[:].tensor, offset=lq0 * L,
                ap=[[0, D], [1, G * L]]))
            return t_zq

        def emit_acts(lq0, G, t_t):
            t_a = apool.tile([D, G, L], BF16, tag="a", name=f"a{lq0}")
            nc.scalar.activation(t_a[:], t_t[:], AF.Tanh)
            t_e = epool.tile([D, G, L], BF16, tag="e", name=f"e{lq0}")
            nc.scalar.activation(t_e[:], t_a[:], AF.Exp, scale=CVAL)
            return t_e

        t_ss = singles.tile([D, 2 * len(PARTS)], F32)

        def emit_epilogue(h):
            """s = num/(den+fb) + fb*hmean, gate, fuse, source2token partials
            for the queries of part h. Branch pairs emitted
            phase-by-phase so each engine's in-order stream overlaps."""
            qlo = pof[h]
            W = PARTS[h]
            t_s, t_den2, t_rec, t_f, t_en, t_d, t_m2, p_g = (
                {}, {}, {}, {}, {}, {}, {}, {})
            nd = [(t_numF[h], t_denF[h]), (t_numB[h], t_denB[h])]
            fbs = [t_fb3[:, qlo:qlo + W], t_fb3[:, Q + qlo:Q + qlo + W]]
            hq = t_h[:, qlo:qlo + W]
            for bi in range(2):
                t_den2[bi] = work.tile([D, W], F32, tag=f"den2{bi}",
                                       name=f"t_den2{bi}_{h}")
                nc.gpsimd.tensor_add(t_den2[bi][:], nd[bi][1][:], fbs[bi])
            for bi in range(2):
                t_rec[bi] = work.tile([D, W], F32, tag=f"rec{bi}",
                                      name=f"t_rec{bi}_{h}")
                nc.vector.reciprocal(t_rec[bi][:], t_den2[bi][:])
            for bi in range(2):
                t_s[bi] = work.tile([D, W], F32, tag=f"s{bi}",
                                    name=f"t_s{bi}_{h}")
                nc.gpsimd.tensor_mul(t_s[bi][:], nd[bi][0][:], t_rec[bi][:])
            for bi in range(2):
                nc.vector.scalar_tensor_tensor(
                    out=t_s[bi][:], in0=fbs[bi], scalar=t_hm[:, 0:1],
                    in1=t_s[bi][:], op0=ALU.mult, op1=ALU.add)  # += fb*hmean
            for bi in range(2):
                p_g[bi] = psum.tile([D, W], F32, tag="ph", name=f"p_g{bi}_{h}")
                nc.tensor.matmul(p_g[bi][:], t_Wf1, t_s[bi][:],
                                 start=True, stop=False)
                nc.tensor.matmul(p_g[bi][:], t_Wf2, hq,
                                 start=False, stop=True)
            for bi in range(2):
                # sigmoid via exp (keeps every activation in one func set)
                t_en[bi] = work.tile([D, W], F32, tag=f"gen{bi}",
                                     name=f"t_en{bi}_{h}")
                nc.scalar.activation(t_en[bi][:], p_g[bi][:], AF.Exp,
                                     scale=-1.0, bias=t_Wf2bn)
            for bi in range(2):
                t_f[bi] = work.tile([D, W], F32, tag=f"f{bi}",
                                    name=f"t_f{bi}_{h}")
                nc.vector.tensor_scalar(
                    out=t_f[bi][:], in0=t_en[bi][:], scalar1=1.0,
                    scalar2=None, op0=ALU.add)
                nc.vector.reciprocal(t_f[bi][:], t_f[bi][:])
            for bi in range(2):
                t_d[bi] = work.tile([D, W], F32, tag=f"d{bi}",
                                    name=f"t_d{bi}_{h}")
                nc.gpsimd.tensor_sub(t_d[bi][:], hq, t_s[bi][:])
            for bi in range(2):
                t_m2[bi] = work.tile([D, W], F32, tag=f"m2{bi}",
                                     name=f"t_m2{bi}_{h}")
                nc.gpsimd.tensor_mul(t_m2[bi][:], t_f[bi][:], t_d[bi][:])
            t_u = {}
            for bi in range(2):
                t_u[bi] = work.tile([D, W], F32, tag=f"u{bi}",
                                    name=f"t_u{bi}_{h}")
                nc.gpsimd.tensor_add(t_u[bi][:], t_s[bi][:], t_m2[bi][:])

            # att_s = elu(u @ Ws1 + Ws1_b) @ Ws + Ws_b ; u feature-split
            p_v, t_v, v_rl, v_nm, v_en = {}, {}, {}, {}, {}
            for j in range(2):
                p_v[j] = psum.tile([D, W], F32, tag="ph", name=f"p_v{j}_{h}")
                nc.tensor.matmul(p_v[j][:], t_Ws1_0[:, j * D:(j + 1) * D],
                                 t_u[0][:], start=True, stop=False)
                nc.tensor.matmul(p_v[j][:], t_Ws1_1[:, j * D:(j + 1) * D],
                                 t_u[1][:], start=False, stop=True)
            for j in range(2):
                v_rl[j] = work.tile([D, W], F32, tag=f"vrl{j}",
                                    name=f"v_rl{j}_{h}")
                nc.scalar.activation(v_rl[j][:], p_v[j][:], AF.Relu,
                                     bias=t_Ws1b[:, j:j + 1])
            for j in range(2):
                v_nm[j] = work.tile([D, W], F32, tag=f"vnm{j}",
                                    name=f"v_nm{j}_{h}")
                nc.vector.tensor_scalar(
                    out=v_nm[j][:], in0=p_v[j][:], scalar1=t_Ws1b[:, j:j + 1],
                    scalar2=0.0, op0=ALU.add, op1=ALU.min)
            for j in range(2):
                v_en[j] = work.tile([D, W], F32, tag=f"ven{j}",
                                    name=f"v_en{j}_{h}")
                nc.scalar.activation(v_en[j][:], v_nm[j][:], AF.Exp)
            for j in range(2):
                t_v[j] = work.tile([D, W], F32, tag=f"v{j}", name=f"t_v{j}_{h}")
                nc.vector.scalar_tensor_tensor(
                    out=t_v[j][:], in0=v_rl[j][:], scalar=-1.0, in1=v_en[j][:],
                    op0=ALU.add, op1=ALU.add)

            p_as, t_as = {}, {}
            for j in range(2):
                p_as[j] = psum.tile([D, W], F32, tag="ph", name=f"p_as{j}_{h}")
                nc.tensor.matmul(p_as[j][:], t_Ws_0[:, j * D:(j + 1) * D],
                                 t_v[0][:], start=True, stop=False)
                nc.tensor.matmul(p_as[j][:], t_Ws_1[:, j * D:(j + 1) * D],
                                 t_v[1][:], start=False, stop=True)
            for j in range(2):
                t_as[j] = work.tile([D, W], F32, tag=f"as{j}",
                                    name=f"t_as{j}_{h}")
                nc.vector.tensor_add(t_as[j][:], p_as[j][:],
                                     _free_bcast(t_Wsb[:, j:j + 1], W))
            for j in range(2):
                t_pscr = work.tile([D, W], F32, tag=f"scrp{j}",
                                   name=f"t_pscr{j}_{h}")
                nc.vector.scalar_tensor_tensor(
                    out=t_pscr[:], in0=t_u[j][:], scalar=1.0, in1=t_as[j][:],
                    op0=ALU.mult, op1=ALU.mult,
                    accum_out=t_ss[:, 2 * h + j:2 * h + j + 1])
            # ship this part's partials immediately; only the last part's
            # small DMA trails the loop
            nc.sync.dma_start(out=d_out[:, 2 * h:2 * h + 2],
                              in_=t_ss[:, 2 * h:2 * h + 2])

        # prefetch all zq-mask DMAs right away (pure inputs; the tile pool's
        # buffer rotation throttles them against consumer progress)
        lq0 = 0
        zqs = {}
        sched = []
        for G in GROUPS:
            zqs[lq0] = emit_zq_dma(lq0, G)
            sched.append((lq0, G))
            lq0 += G

        # software-pipeline: emit each group's Pool add two groups early so
        # Pool works while ACT runs the previous groups' tanh/exp
        adds = {}
        for g in sched[:2]:
            adds[g[0]] = emit_add(g[0], g[1])
        for gi, (lq0, G) in enumerate(sched):
            if gi + 2 < len(sched):
                nlq0, nG = sched[gi + 2]
                adds[nlq0] = emit_add(nlq0, nG)
            t_e = emit_acts(lq0, G, adds[lq0])

            # masked exp and its h-weighted product, full rows; each pass is
            # split KDS[gi] queries DVE / rest Pool to balance the engines
            kd = min(KDS[gi], G)
            t_zq = zqs[lq0]
            t_ez = ppool.tile([D, G, L], BF16, tag="ez", name=f"ez{lq0}")
            if kd > 0:
                nc.vector.tensor_mul(t_ez[:, 0:kd, :], t_e[:, 0:kd, :],
                                     t_zq[:, 0:kd, :])
            if kd < G:
                nc.gpsimd.tensor_mul(t_ez[:, kd:G, :], t_e[:, kd:G, :],
                                     t_zq[:, kd:G, :])
            t_ezh = ppool.tile([D, G, L], BF16, tag="ezh", name=f"ezh{lq0}")
            if kd > 0:
                hb_d = bass.AP(tensor=hb_src.tensor, offset=hb_src.offset,
                               ap=[hb_src.ap[0], [0, kd], [1, L]])
                nc.vector.tensor_mul(t_ezh[:, 0:kd, :], t_ez[:, 0:kd, :], hb_d)
            if kd < G:
                hb_p = bass.AP(tensor=hb_src.tensor, offset=hb_src.offset,
                               ap=[hb_src.ap[0], [0, G - kd], [1, L]])
                nc.gpsimd.tensor_mul(t_ezh[:, kd:G, :], t_ez[:, kd:G, :], hb_p)

            # DVE: windowed sums (4x mode: all-bf16 packed SBUF operands)
            for k in range(G):
                lq = lq0 + k
                h = next(i for i in range(len(PARTS)) if lq < pof[i + 1])
                c = lq - pof[h]
                t_sf = scr.tile([D, L], BF16, tag="ezf")
                nc.vector.tensor_scalar(
                    out=t_sf[:, lq + 1:], in0=t_ez[:, k, lq + 1:],
                    scalar1=1.0, scalar2=0.0, op0=ALU.mult, op1=ALU.add,
                    accum_out=t_denF[h][:, c:c + 1])
                if lq > 0:
                    t_sb = scr.tile([D, L], BF16, tag="ezb")
                    nc.vector.tensor_scalar(
                        out=t_sb[:, 0:lq], in0=t_ez[:, k, 0:lq],
                        scalar1=1.0, scalar2=0.0, op0=ALU.mult,
                        op1=ALU.add, accum_out=t_denB[h][:, c:c + 1])
                t_nf = scr.tile([D, L], BF16, tag="scrf")
                nc.vector.tensor_scalar(
                    out=t_nf[:, lq + 1:], in0=t_ezh[:, k, lq + 1:],
                    scalar1=1.0, scalar2=0.0, op0=ALU.mult, op1=ALU.add,
                    accum_out=t_numF[h][:, c:c + 1])
                if lq > 0:
                    t_nb = scr.tile([D, L], BF16, tag="scrb")
                    nc.vector.tensor_scalar(
                        out=t_nb[:, 0:lq], in0=t_ezh[:, k, 0:lq],
                        scalar1=1.0, scalar2=0.0, op0=ALU.mult,
                        op1=ALU.add, accum_out=t_numB[h][:, c:c + 1])

            if lq0 + G in pof[1:-1]:
                # this part's queries are complete: its epilogue overlaps
                # the remaining loop groups
                emit_epilogue(pof.index(lq0 + G) - 1)

        emit_epilogue(len(PARTS) - 1)

    nc.compile()
    return nc


def _get_nc():
    if "nc" not in _CACHE:
        _CACHE["nc"] = _build_program()
    return _CACHE["nc"]


def _host_prep(mask, h_all, W1s, W2s, bs):
    per_core = []
    for c in range(NCORES):
        b, half = divmod(c, 2)
        # even half: natural token order; odd half: fully reversed. In both
        # cases this core's queries sit at positions 0..Q-1 and the
        # branch windows are position slices [0,lq) / (lq,200).
        perm = np.arange(L) if half == 0 else np.arange(L - 1, -1, -1)
        gq = perm[:Q]                            # global id of query at pos lq
        hc = h_all[b][perm]                      # [L, D] by position
        hb_c = np.ascontiguousarray(hc.T.astype(ml_dtypes.bfloat16))
        h1s = hc[:Q] @ W1s                       # [Q, D]
        h2bs = hc @ W2s + bs                     # [L, D]
        hm = hc.mean(axis=0)                     # [D]
        hf1_c = np.ascontiguousarray(np.concatenate(
            [h1s.T, h2bs.T], axis=1).astype(np.float32))
        hf2_c = np.ascontiguousarray(np.concatenate(
            [hc.T, hm[:, None]], axis=1).astype(np.float32))
        assert hf1_c.shape == (D, PH1_W) and hf2_c.shape == (D, PH2_W)
        mk = mask[b][perm]                       # key padness by position [L]
        mq = mask[b][gq]                         # query padness [Q]
        pm = perm[None, :]                       # global key id per position
        padbad = mk[None, :] & ~mq[:, None]      # [Q, L]
        allow_fw = ~padbad & (pm > gq[:, None])
        allow_bw = ~padbad & (pm < gq[:, None])
        zF = allow_fw if half == 0 else allow_bw   # window (lq, 200)
        zP = allow_bw if half == 0 else allow_fw   # window [0, lq)
        fbF = (~zF.any(axis=1)).astype(np.float32)
        fbP = (~zP.any(axis=1)).astype(np.float32)
        fb_row = np.ascontiguousarray(np.concatenate(
            [fbF, fbP])[None, :], dtype=np.float32)
        # effective key mask per query: pad queries attend everywhere
        zq = np.where(mq[:, None], np.float32(1.0),
                      (~mk)[None, :].astype(np.float32))
        zq_row = np.ascontiguousarray(
            zq.reshape(1, Q * L).astype(ml_dtypes.bfloat16))
        per_core.append((hb_c, hf1_c, hf2_c, fb_row, zq_row))
    return per_core


def _prepare_in_maps(inputs):
    f32 = lambda k: np.asarray(inputs[k], dtype=np.float32)
    x = np.asarray(inputs["x"]).astype(np.int64)
    mask = np.asarray(inputs["mask"]).astype(bool)
    emb = f32("emb")

    sig = np.r_[D:2 * D, 0:D]   # swap the fw/bw feature halves
    Ws1_w, Ws_w = f32("Ws1_w"), f32("Ws_w")
    Ws1_b, Ws_b = f32("Ws1_b"), f32("Ws_b")

    # h = elu(xe @ Wh + Wh_b) for all batches (tiny; part of input prep,
    # like the embedding gather)
    xe = emb[x]                                       # [B, L, D]
    pre = xe @ f32("Wh_w") + f32("Wh_b")
    h_all = np.where(pre > 0, pre, np.expm1(np.minimum(pre, 0.0)))
    W1s, W2s = f32("W1_w") / CVAL, f32("W2_w") / CVAL
    bs = f32("b") / CVAL

    def late_for(swap):
        if swap:
            W1, W, b1, bb = (Ws1_w[sig][:, sig], Ws_w[sig][:, sig],
                             Ws1_b[sig], Ws_b[sig])
        else:
            W1, W, b1, bb = Ws1_w, Ws_w, Ws1_b, Ws_b
        cols = [
            f32("Wf1_w"), f32("Wf2_w"),
            W1[0:D, :], W1[D:2 * D, :], W[0:D, :], W[D:2 * D, :],
            f32("Wf2_b").reshape(D, 1),
            b1.reshape(2, D).T, bb.reshape(2, D).T,
            -f32("Wf2_b").reshape(D, 1),
        ]
        p = np.concatenate(cols, axis=1).astype(np.float32)
        assert p.shape == (D, PL_W), p.shape
        return np.ascontiguousarray(p)

    late = [late_for(False), late_for(True)]
    per_core = _host_prep(mask, h_all, W1s, W2s, bs)
    in_maps = []
    for c, (hb_c, hf1_c, hf2_c, fb_row, zq_row) in enumerate(per_core):
        in_maps.append(dict(hb=hb_c, hf1=hf1_c, hf2=hf2_c, late=late[c % 2],
                            fb=fb_row, zq=zq_row))
    return in_maps


def _assemble(res, inputs):
    f32 = lambda k: np.asarray(inputs[k], dtype=np.float32)
    ss = np.zeros((B, 2 * D), np.float32)
    for c in range(NCORES):
        o = res[c]["out"]  # [D, 2P]: per-part [F, P] partials
        oF, oP = o[:, 0::2].sum(axis=1), o[:, 1::2].sum(axis=1)
        if c % 2 == 0:     # branch-F = fw, branch-P = bw
            ss[c // 2] += np.concatenate([oF, oP])
        else:              # swapped
            ss[c // 2] += np.concatenate([oP, oF])

    F1_w, F1_b = f32("F1_w"), f32("F1_b")
    F2_w, F2_b = f32("F2_w"), f32("F2_b")
    out = np.maximum(ss @ F1_w + F1_b, 0.0) @ F2_w + F2_b
    return out.astype(np.float32)


def kernel(**inputs):
    in_maps = _prepare_in_maps(inputs)
    nc = _get_nc()
    res = run_bass_kernel_spmd(nc, in_maps, core_ids=list(range(NCORES))).results
    return _assemble(res, inputs)


# revision 81
# speedup vs baseline: 1.0171x; 1.0171x over previous
"""DiSAN forward kernel on 8 TRN2 NeuronCores (Bass/Tile, SPMD).

Sharding: core c handles batch b = c//2 and query half c%2 (100 queries each).
Per-core token permutation (natural order for even cores, fully reversed for
odd ones) puts the core's queries at positions 0..99 and turns both attention
directions into the position windows [0,lq) / (lq,200), so one program serves
all 8 cores; the fw/bw meaning of the two branches is unscrambled on the host
by swapping weight feature-halves and output halves for odd cores.

The [L,L,D] attention tensor never touches HBM. The O(L*D) prologue
(h = elu(emb[x] @ Wh + b), h1 = (h@W1)/C, h2b = (h@W2+b)/C, hmean) runs on
the host alongside the embedding gather and ships as small packs. On device,
per group of G queries: one Pool add builds the logits (h2b broadcast over
the group + h1 column broadcast over keys; the 1/C tanh prescale is folded
in on the host), ScalarE runs one grouped tanh and one grouped exp (bf16),
then two bf16 product passes - ez = e*zq (zq = per-query effective key
mask, broadcast-DMAd from a host row: key-mask for real queries, all-ones
for pad queries) and ezh = ez*h - split between DVE tensor_tensor (2x mode)
and Pool to balance the engines. Per query, four windowed tensor_scalar
ops with accum_out (DVE 4x mode: all-bf16 packed SBUF operands) reduce
ez/ezh over the compile-time F/B window slices into num/den columns.
Queries with an empty key set (host-detected) carry fb=1; their s falls
back to mean(h), matching the reference's uniform softmax over an all
-1e13 row. The fusion-gate + source2token epilogue runs in two query
parts (75/25) so the first part overlaps the remaining loop without its
ACT ops delaying the loop's exp stream too early; each core
emits per-part partial poolings [D,6] and the host sums them and applies
the tiny final MLP.
"""

import numpy as np
import ml_dtypes
from contextlib import ExitStack

import concourse.bass as bass
import concourse.bacc as bacc
import concourse.tile as tile
from concourse import mybir
from concourse.bass_utils import run_bass_kernel_spmd

B, L, D, NCLS = 4, 200, 100, 20
Q = 100           # queries per core
NCORES = 8
CVAL = 5.0
F32 = mybir.dt.float32
BF16 = mybir.dt.bfloat16
AF = mybir.ActivationFunctionType
ALU = mybir.AluOpType

_CACHE = {}

# query-group sizes for the main loop (sum = Q); ramped ends so the first
# DVE/Pool work arrives early and the last group drains quickly. Queries
# in each finished part get their epilogue emitted early, overlapping the
# remaining loop. Within each group the ez/ezh product passes are split
# ~25% DVE (tensor_tensor 2x bf16, ~0.52ns/el) / 75% Pool (~0.83ns/el) so
# the per-group flows through DVE and Pool match; the logits adds always
# run on Pool (their h1-broadcast operand has innermost stride 0, which
# disqualifies the DVE fast modes) and DVE carries the fixed
# windowed-reduction load.
GROUPS = (1, 2, 3, 6, 10, 13, 15, 13, 12, 13, 12)
# epilogue part sizes; parts end at query boundaries 75 / 100, which
# must coincide with group boundaries so each part's epilogue can be
# emitted as soon as its queries finish
PARTS = (75, 25)
# per-group count of product-pass queries handled by DVE (rest on Pool)
KDS = (1, 2, 3, 6, 5, 4, 3, 3, 2, 2, 1)

# f32 packs of host-computed per-core activations. hf1 (h1s | h2bs) heads
# the whole pipeline, so it ships separately from hf2 (hT | hmean), which
# is only needed by the later product/epilogue stages.
PH1 = dict(H1=0, H2B=Q)
PH1_W = Q + L
PH2 = dict(H=0, HM=L)
PH2_W = L + 1
# f32 late pack: gate + source2token weights
PL = dict(WF1=0, WF2=100, WS1_0=200, WS1_1=400, WS_0=600, WS_1=800,
          WF2B=1000, WS1B=1001, WSB=1003, WF2BN=1005)
PL_W = 1006


def _free_bcast(ap, n):
    """Broadcast a [P,1] AP along the free dim to [P,n] with stride 0."""
    return bass.AP(tensor=ap.tensor, offset=ap.offset, ap=[ap.ap[0], [0, n]])


def _build_program():
    nc = bacc.Bacc()
    d_hb = nc.declare_dram_parameter("hb", [D, L], BF16, isOutput=False)
    d_hf1 = nc.declare_dram_parameter("hf1", [D, PH1_W], F32, isOutput=False)
    d_hf2 = nc.declare_dram_parameter("hf2", [D, PH2_W], F32, isOutput=False)
    d_late = nc.declare_dram_parameter("late", [D, PL_W], F32, isOutput=False)
    # per-query effective key mask rows: ones if query is PAD else key-mask
    d_zq = nc.declare_dram_parameter("zq", [1, Q * L], BF16, isOutput=False)
    # fb row: fbF | fbP  ({0,1} f32)
    d_fb = nc.declare_dram_parameter("fb", [1, 2 * Q], F32, isOutput=False)
    # per-part partial source2token poolings: [F_p0, P_p0, F_p1, P_p1, ...]
    d_out = nc.declare_dram_parameter("out", [D, 6], F32, isOutput=True)

    with tile.TileContext(nc) as tc, ExitStack() as ctx:
        singles = ctx.enter_context(tc.tile_pool(name="singles", bufs=1))
        work = ctx.enter_context(tc.tile_pool(name="work", bufs=3))
        psum = ctx.enter_context(tc.tile_pool(name="psum", bufs=4, space="PSUM"))
        tpool = ctx.enter_context(tc.tile_pool(name="tpool", bufs=3))
        apool = ctx.enter_context(tc.tile_pool(name="apool", bufs=3))
        epool = ctx.enter_context(tc.tile_pool(name="epool", bufs=4))
        zpool = ctx.enter_context(tc.tile_pool(name="zpool", bufs=3))
        ppool = ctx.enter_context(tc.tile_pool(name="ppool", bufs=4))
        scr = ctx.enter_context(tc.tile_pool(name="scr", bufs=3))

        # hf1 first: the h1s/h2bs it carries head the whole pipeline
        t_hf1 = singles.tile([D, PH1_W], F32, tag="hf1")
        nc.sync.dma_start(out=t_hf1[:], in_=d_hf1[:])
        t_hbt = singles.tile([D, L], BF16, tag="hbt")
        nc.sync.dma_start(out=t_hbt[:], in_=d_hb[:])
        t_hf2 = singles.tile([D, PH2_W], F32, tag="hf2")
        nc.sync.dma_start(out=t_hf2[:], in_=d_hf2[:])
        # fbF | fbP rows broadcast across partitions
        t_fb3 = singles.tile([D, 2 * Q], F32, tag="fb3")
        nc.sync.dma_start(out=t_fb3[:], in_=bass.AP(
            tensor=d_fb[:].tensor, offset=0, ap=[[0, D], [1, 2 * Q]]))
        t_late = singles.tile([D, PL_W], F32, tag="late")
        nc.gpsimd.dma_start(out=t_late[:], in_=d_late[:])

        t_h = t_hf2[:, PH2["H"]:PH2["H"] + L]
        t_h1 = t_hf1[:, PH1["H1"]:PH1["H1"] + Q]
        t_h2b = t_hf1[:, PH1["H2B"]:PH1["H2B"] + L]
        t_hm = t_hf2[:, PH2["HM"]:PH2["HM"] + 1]
        t_Wf1 = t_late[:, PL["WF1"]:PL["WF1"] + D]
        t_Wf2 = t_late[:, PL["WF2"]:PL["WF2"] + D]
        t_Ws1_0 = t_late[:, PL["WS1_0"]:PL["WS1_0"] + 2 * D]
        t_Ws1_1 = t_late[:, PL["WS1_1"]:PL["WS1_1"] + 2 * D]
        t_Ws_0 = t_late[:, PL["WS_0"]:PL["WS_0"] + 2 * D]
        t_Ws_1 = t_late[:, PL["WS_1"]:PL["WS_1"] + 2 * D]
        t_Wf2b = t_late[:, PL["WF2B"]:PL["WF2B"] + 1]
        t_Ws1b = t_late[:, PL["WS1B"]:PL["WS1B"] + 2]
        t_Wsb = t_late[:, PL["WSB"]:PL["WSB"] + 2]
        t_Wf2bn = t_late[:, PL["WF2BN"]:PL["WF2BN"] + 1]

        t_ones = singles.tile([1, D], F32)
        nc.vector.memset(t_ones[:], 1.0)
        # warm the ACT function-set table load (1.3us) during the input DMAs
        t_warm = singles.tile([1, 1], F32, tag="warm")
        nc.scalar.activation(t_warm[:], t_ones[0:1, 0:1], AF.Exp)

        # separate accumulator tiles per epilogue part so each part's
        # epilogue only depends on the groups that wrote it
        pof = [0]
        for w in PARTS:
            pof.append(pof[-1] + w)
        t_numF, t_denF, t_numB, t_denB = {}, {}, {}, {}
        for h, w in enumerate(PARTS):
            t_numF[h] = singles.tile([D, w], F32, name=f"t_numF{h}")
            t_denF[h] = singles.tile([D, w], F32, name=f"t_denF{h}")
            t_numB[h] = singles.tile([D, w], F32, name=f"t_numB{h}")
            t_denB[h] = singles.tile([D, w], F32, name=f"t_denB{h}")

        # zero the columns that sliced-window skipping never writes
        nc.gpsimd.memset(t_numB[0][:, 0:1], 0.0)
        nc.gpsimd.memset(t_denB[0][:, 0:1], 0.0)

        h2b_src = t_h2b
        hb_src = t_hbt[:]

        def emit_add(lq0, G):
            # t[d, k, m] = h2bs[d, m] + h1s[d, lq0+k]  (Pool)
            h2b_grp = bass.AP(tensor=h2b_src.tensor, offset=h2b_src.offset,
                              ap=[h2b_src.ap[0], [0, G], [1, L]])
            h1c = t_h1[:, lq0:lq0 + G]
            h1_grp = bass.AP(tensor=h1c.tensor, offset=h1c.offset,
                             ap=[h1c.ap[0], h1c.ap[1], [0, L]])
            t_t = tpool.tile([D, G, L], F32, tag="t", name=f"t{lq0}")
            nc.gpsimd.tensor_add(t_t[:], h2b_grp, h1_grp)
            return t_t

        def emit_zq_dma(lq0, G):
            t_zq = zpool.tile([D, G, L], BF16, tag="zq", name=f"zq{lq0}")
            nc.sync.dma_start(out=t_zq[:], in_=bass.AP(
                tensor=d_zq[:].tensor, offset=lq0 * L,
                ap=[[0, D], [1, G * L]]))
            return t_zq

        def emit_acts(lq0, G, t_t):
            t_a = apool.tile([D, G, L], BF16, tag="a", name=f"a{lq0}")
            nc.scalar.activation(t_a[:], t_t[:], AF.Tanh)
            t_e = epool.tile([D, G, L], BF16, tag="e", name=f"e{lq0}")
            nc.scalar.activation(t_e[:], t_a[:], AF.Exp, scale=CVAL)
            return t_e

        t_ss = singles.tile([D, 2 * len(PARTS)], F32)

        def emit_epilogue(h):
            """s = num/(den+fb) + fb*hmean, gate, fuse, source2token partials
            for the queries of part h. Branch pairs emitted
            phase-by-phase so each engine's in-order stream overlaps."""
            qlo = pof[h]
            W = PARTS[h]
            t_s, t_den2, t_rec, t_f, t_en, t_d, t_m2, p_g = (
                {}, {}, {}, {}, {}, {}, {}, {})
            nd = [(t_numF[h], t_denF[h]), (t_numB[h], t_denB[h])]
            fbs = [t_fb3[:, qlo:qlo + W], t_fb3[:, Q + qlo:Q + qlo + W]]
            hq = t_h[:, qlo:qlo + W]
            for bi in range(2):
                t_den2[bi] = work.tile([D, W], F32, tag=f"den2{bi}",
                                       name=f"t_den2{bi}_{h}")
                nc.gpsimd.tensor_add(t_den2[bi][:], nd[bi][1][:], fbs[bi])
            for bi in range(2):
                t_rec[bi] = work.tile([D, W], F32, tag=f"rec{bi}",
                                      name=f"t_rec{bi}_{h}")
                nc.vector.reciprocal(t_rec[bi][:], t_den2[bi][:])
            for bi in range(2):
                t_s[bi] = work.tile([D, W], F32, tag=f"s{bi}",
                                    name=f"t_s{bi}_{h}")
                nc.gpsimd.tensor_mul(t_s[bi][:], nd[bi][0][:], t_rec[bi][:])
            for bi in range(2):
                nc.vector.scalar_tensor_tensor(
                    out=t_s[bi][:], in0=fbs[bi], scalar=t_hm[:, 0:1],
                    in1=t_s[bi][:], op0=ALU.mult, op1=ALU.add)  # += fb*hmean
            for bi in range(2):
                p_g[bi] = psum.tile([D, W], F32, tag="ph", name=f"p_g{bi}_{h}")
                nc.tensor.matmul(p_g[bi][:], t_Wf1, t_s[bi][:],
                                 start=True, stop=False)
                nc.tensor.matmul(p_g[bi][:], t_Wf2, hq,
                                 start=False, stop=True)
            for bi in range(2):
                # sigmoid via exp (keeps every activation in one func set)
                t_en[bi] = work.tile([D, W], F32, tag=f"gen{bi}",
                                     name=f"t_en{bi}_{h}")
                nc.scalar.activation(t_en[bi][:], p_g[bi][:], AF.Exp,
                                     scale=-1.0, bias=t_Wf2bn)
            for bi in range(2):
                t_f[bi] = work.tile([D, W], F32, tag=f"f{bi}",
                                    name=f"t_f{bi}_{h}")
                nc.vector.tensor_scalar(
                    out=t_f[bi][:], in0=t_en[bi][:], scalar1=1.0,
                    scalar2=None, op0=ALU.add)
                nc.vector.reciprocal(t_f[bi][:], t_f[bi][:])
            for bi in range(2):
                t_d[bi] = work.tile([D, W], F32, tag=f"d{bi}",
                                    name=f"t_d{bi}_{h}")
                nc.gpsimd.tensor_sub(t_d[bi][:], hq, t_s[bi][:])
            for bi in range(2):
                t_m2[bi] = work.tile([D, W], F32, tag=f"m2{bi}",
                                     name=f"t_m2{bi}_{h}")
                nc.gpsimd.tensor_mul(t_m2[bi][:], t_f[bi][:], t_d[bi][:])
            t_u = {}
            for bi in range(2):
                t_u[bi] = work.tile([D, W], F32, tag=f"u{bi}",
                                    name=f"t_u{bi}_{h}")
                nc.gpsimd.tensor_add(t_u[bi][:], t_s[bi][:], t_m2[bi][:])

            # att_s = elu(u @ Ws1 + Ws1_b) @ Ws + Ws_b ; u feature-split
            p_v, t_v, v_rl, v_nm, v_en = {}, {}, {}, {}, {}
            for j in range(2):
                p_v[j] = psum.tile([D, W], F32, tag="ph", name=f"p_v{j}_{h}")
                nc.tensor.matmul(p_v[j][:], t_Ws1_0[:, j * D:(j + 1) * D],
                                 t_u[0][:], start=True, stop=False)
                nc.tensor.matmul(p_v[j][:], t_Ws1_1[:, j * D:(j + 1) * D],
                                 t_u[1][:], start=False, stop=True)
            for j in range(2):
                v_rl[j] = work.tile([D, W], F32, tag=f"vrl{j}",
                                    name=f"v_rl{j}_{h}")
                nc.scalar.activation(v_rl[j][:], p_v[j][:], AF.Relu,
                                     bias=t_Ws1b[:, j:j + 1])
            for j in range(2):
                v_nm[j] = work.tile([D, W], F32, tag=f"vnm{j}",
                                    name=f"v_nm{j}_{h}")
                nc.vector.tensor_scalar(
                    out=v_nm[j][:], in0=p_v[j][:], scalar1=t_Ws1b[:, j:j + 1],
                    scalar2=0.0, op0=ALU.add, op1=ALU.min)
            for j in range(2):
                v_en[j] = work.tile([D, W], F32, tag=f"ven{j}",
                                    name=f"v_en{j}_{h}")
                nc.scalar.activation(v_en[j][:], v_nm[j][:], AF.Exp)
            for j in range(2):
                t_v[j] = work.tile([D, W], F32, tag=f"v{j}", name=f"t_v{j}_{h}")
                nc.vector.scalar_tensor_tensor(
                    out=t_v[j][:], in0=v_rl[j][:], scalar=-1.0, in1=v_en[j][:],
                    op0=ALU.add, op1=ALU.add)

            p_as, t_as = {}, {}
            for j in range(2):
                p_as[j] = psum.tile([D, W], F32, tag="ph", name=f"p_as{j}_{h}")
                nc.tensor.matmul(p_as[j][:], t_Ws_0[:, j * D:(j + 1) * D],
                                 t_v[0][:], start=True, stop=False)
                nc.tensor.matmul(p_as[j][:], t_Ws_1[:, j * D:(j + 1) * D],
                                 t_v[1][:], start=False, stop=True)
            for j in range(2):
                t_as[j] = work.tile([D, W], F32, tag=f"as{j}",
                                    name=f"t_as{j}_{h}")
                nc.vector.tensor_add(t_as[j][:], p_as[j][:],
                                     _free_bcast(t_Wsb[:, j:j + 1], W))
            for j in range(2):
                t_pscr = work.tile([D, W], F32, tag=f"scrp{j}",
                                   name=f"t_pscr{j}_{h}")
                nc.vector.scalar_tensor_tensor(
                    out=t_pscr[:], in0=t_u[j][:], scalar=1.0, in1=t_as[j][:],
                    op0=ALU.mult, op1=ALU.mult,
                    accum_out=t_ss[:, 2 * h + j:2 * h + j + 1])
            # ship this part's partials immediately; only the last part's
            # small DMA trails the loop
            nc.sync.dma_start(out=d_out[:, 2 * h:2 * h + 2],
                              in_=t_ss[:, 2 * h:2 * h + 2])

        # prefetch all zq-mask DMAs right away (pure inputs; the tile pool's
        # buffer rotation throttles them against consumer progress)
        lq0 = 0
        zqs = {}
        sched = []
        for G in GROUPS:
            zqs[lq0] = emit_zq_dma(lq0, G)
            sched.append((lq0, G))
            lq0 += G

        # software-pipeline: emit each group's Pool add two groups early so
        # Pool works while ACT runs the previous groups' tanh/exp
        adds = {}
        for g in sched[:2]:
            adds[g[0]] = emit_add(g[0], g[1])
        for gi, (lq0, G) in enumerate(sched):
            if gi + 2 < len(sched):
                nlq0, nG = sched[gi + 2]
                adds[nlq0] = emit_add(nlq0, nG)
            t_e = emit_acts(lq0, G, adds[lq0])

            # masked exp and its h-weighted product, full rows; each pass is
            # split KDS[gi] queries DVE / rest Pool to balance the engines
            kd = min(KDS[gi], G)
            t_zq = zqs[lq0]
            t_ez = ppool.tile([D, G, L], BF16, tag="ez", name=f"ez{lq0}")
            if kd > 0:
                nc.vector.tensor_mul(t_ez[:, 0:kd, :], t_e[:, 0:kd, :],
                                     t_zq[:, 0:kd, :])
            if kd < G:
                nc.gpsimd.tensor_mul(t_ez[:, kd:G, :], t_e[:, kd:G, :],
                                     t_zq[:, kd:G, :])
            t_ezh = ppool.tile([D, G, L], BF16, tag="ezh", name=f"ezh{lq0}")
            if kd > 0:
                hb_d = bass.AP(tensor=hb_src.tensor, offset=hb_src.offset,
                               ap=[hb_src.ap[0], [0, kd], [1, L]])
                nc.vector.tensor_mul(t_ezh[:, 0:kd, :], t_ez[:, 0:kd, :], hb_d)
            if kd < G:
                hb_p = bass.AP(tensor=hb_src.tensor, offset=hb_src.offset,
                               ap=[hb_src.ap[0], [0, G - kd], [1, L]])
                nc.gpsimd.tensor_mul(t_ezh[:, kd:G, :], t_ez[:, kd:G, :], hb_p)

            # DVE: windowed sums (4x mode: all-bf16 packed SBUF operands)
            for k in range(G):
                lq = lq0 + k
                h = next(i for i in range(len(PARTS)) if lq < pof[i + 1])
                c = lq - pof[h]
                t_sf = scr.tile([D, L], BF16, tag="ezf")
                nc.vector.tensor_scalar(
                    out=t_sf[:, lq + 1:], in0=t_ez[:, k, lq + 1:],
                    scalar1=1.0, scalar2=0.0, op0=ALU.mult, op1=ALU.add,
                    accum_out=t_denF[h][:, c:c + 1])
                if lq > 0:
                    t_sb = scr.tile([D, L], BF16, tag="ezb")
                    nc.vector.tensor_scalar(
                        out=t_sb[:, 0:lq], in0=t_ez[:, k, 0:lq],
                        scalar1=1.0, scalar2=0.0, op0=ALU.mult,
                        op1=ALU.add, accum_out=t_denB[h][:, c:c + 1])
                t_nf = scr.tile([D, L], BF16, tag="scrf")
                nc.vector.tensor_scalar(
                    out=t_nf[:, lq + 1:], in0=t_ezh[:, k, lq + 1:],
                    scalar1=1.0, scalar2=0.0, op0=ALU.mult, op1=ALU.add,
                    accum_out=t_numF[h][:, c:c + 1])
                if lq > 0:
                    t_nb = scr.tile([D, L], BF16, tag="scrb")
                    nc.vector.tensor_scalar(
                        out=t_nb[:, 0:lq], in0=t_ezh[:, k, 0:lq],
                        scalar1=1.0, scalar2=0.0, op0=ALU.mult,
                        op1=ALU.add, accum_out=t_numB[h][:, c:c + 1])

            if lq0 + G in pof[1:-1]:
                # this part's queries are complete: its epilogue overlaps
                # the remaining loop groups
                emit_epilogue(pof.index(lq0 + G) - 1)

        emit_epilogue(len(PARTS) - 1)

    nc.compile()
    return nc


def _get_nc():
    if "nc" not in _CACHE:
        _CACHE["nc"] = _build_program()
    return _CACHE["nc"]


def _host_prep(mask, h_all, W1s, W2s, bs):
    per_core = []
    for c in range(NCORES):
        b, half = divmod(c, 2)
        # even half: natural token order; odd half: fully reversed. In both
        # cases this core's queries sit at positions 0..Q-1 and the
        # branch windows are position slices [0,lq) / (lq,200).
        perm = np.arange(L) if half == 0 else np.arange(L - 1, -1, -1)
        gq = perm[:Q]                            # global id of query at pos lq
        hc = h_all[b][perm]                      # [L, D] by position
        hb_c = np.ascontiguousarray(hc.T.astype(ml_dtypes.bfloat16))
        h1s = hc[:Q] @ W1s                       # [Q, D]
        h2bs = hc @ W2s + bs                     # [L, D]
        hm = hc.mean(axis=0)                     # [D]
        hf1_c = np.ascontiguousarray(np.concatenate(
            [h1s.T, h2bs.T], axis=1).astype(np.float32))
        hf2_c = np.ascontiguousarray(np.concatenate(
            [hc.T, hm[:, None]], axis=1).astype(np.float32))
        assert hf1_c.shape == (D, PH1_W) and hf2_c.shape == (D, PH2_W)
        mk = mask[b][perm]                       # key padness by position [L]
        mq = mask[b][gq]                         # query padness [Q]
        pm = perm[None, :]                       # global key id per position
        padbad = mk[None, :] & ~mq[:, None]      # [Q, L]
        allow_fw = ~padbad & (pm > gq[:, None])
        allow_bw = ~padbad & (pm < gq[:, None])
        zF = allow_fw if half == 0 else allow_bw   # window (lq, 200)
        zP = allow_bw if half == 0 else allow_fw   # window [0, lq)
        fbF = (~zF.any(axis=1)).astype(np.float32)
        fbP = (~zP.any(axis=1)).astype(np.float32)
        fb_row = np.ascontiguousarray(np.concatenate(
            [fbF, fbP])[None, :], dtype=np.float32)
        # effective key mask per query: pad queries attend everywhere
        zq = np.where(mq[:, None], np.float32(1.0),
                      (~mk)[None, :].astype(np.float32))
        zq_row = np.ascontiguousarray(
            zq.reshape(1, Q * L).astype(ml_dtypes.bfloat16))
        per_core.append((hb_c, hf1_c, hf2_c, fb_row, zq_row))
    return per_core


def _prepare_in_maps(inputs):
    f32 = lambda k: np.asarray(inputs[k], dtype=np.float32)
    x = np.asarray(inputs["x"]).astype(np.int64)
    mask = np.asarray(inputs["mask"]).astype(bool)
    emb = f32("emb")

    sig = np.r_[D:2 * D, 0:D]   # swap the fw/bw feature halves
    Ws1_w, Ws_w = f32("Ws1_w"), f32("Ws_w")
    Ws1_b, Ws_b = f32("Ws1_b"), f32("Ws_b")

    # h = elu(xe @ Wh + Wh_b) for all batches (tiny; part of input prep,
    # like the embedding gather)
    xe = emb[x]                                       # [B, L, D]
    pre = xe @ f32("Wh_w") + f32("Wh_b")
    h_all = np.where(pre > 0, pre, np.expm1(np.minimum(pre, 0.0)))
    W1s, W2s = f32("W1_w") / CVAL, f32("W2_w") / CVAL
    bs = f32("b") / CVAL

    def late_for(swap):
        if swap:
            W1, W, b1, bb = (Ws1_w[sig][:, sig], Ws_w[sig][:, sig],
                             Ws1_b[sig], Ws_b[sig])
        else:
            W1, W, b1, bb = Ws1_w, Ws_w, Ws1_b, Ws_b
        cols = [
            f32("Wf1_w"), f32("Wf2_w"),
            W1[0:D, :], W1[D:2 * D, :], W[0:D, :], W[D:2 * D, :],
            f32("Wf2_b").reshape(D, 1),
            b1.reshape(2, D).T, bb.reshape(2, D).T,
            -f32("Wf2_b").reshape(D, 1),
        ]
        p = np.concatenate(cols, axis=1).astype(np.float32)
        assert p.shape == (D, PL_W), p.shape
        return np.ascontiguousarray(p)

    late = [late_for(False), late_for(True)]
    per_core = _host_prep(mask, h_all, W1s, W2s, bs)
    in_maps = []
    for c, (hb_c, hf1_c, hf2_c, fb_row, zq_row) in enumerate(per_core):
        in_maps.append(dict(hb=hb_c, hf1=hf1_c, hf2=hf2_c, late=late[c % 2],
                            fb=fb_row, zq=zq_row))
    return in_maps


def _assemble(res, inputs):
    f32 = lambda k: np.asarray(inputs[k], dtype=np.float32)
    ss = np.zeros((B, 2 * D), np.float32)
    for c in range(NCORES):
        o = res[c]["out"]  # [D, 2P]: per-part [F, P] partials
        oF, oP = o[:, 0::2].sum(axis=1), o[:, 1::2].sum(axis=1)
        if c % 2 == 0:     # branch-F = fw, branch-P = bw
            ss[c // 2] += np.concatenate([oF, oP])
        else:              # swapped
            ss[c // 2] += np.concatenate([oP, oF])

    F1_w, F1_b = f32("F1_w"), f32("F1_b")
    F2_w, F2_b = f32("F2_w"), f32("F2_b")
    out = np.maximum(ss @ F1_w + F1_b, 0.0) @ F2_w + F2_b
    return out.astype(np.float32)


def kernel(**inputs):
    in_maps = _prepare_in_maps(inputs)
    nc = _get_nc()
    res = run_bass_kernel_spmd(nc, in_maps, core_ids=list(range(NCORES))).results
    return _assemble(res, inputs)
